# revision 1
# baseline (speedup 1.0000x reference)
# Trainium2 Bass kernel for EndPointRepr (span endpoint representations).
#
# reference:
#   h = encoded_input @ W + b                    # [B, S, P]
#   res_k[q] = concat(h[qb[q], s_k[q]], h[qb[q], e_k[q]]) * (e_k[q] >= s_k[q])
#
# Sharding: data-parallel over batch. Core c owns batch c; the host routes
# each query to its batch's core. Invalid queries (e < s) point at a zeroed
# pad row of h, so validity masking costs nothing on device.
#
# Device pipeline (fp32 end to end):
#   phase 1: per 128-row block, PE-transpose X tiles (k onto partitions),
#            matmul against W k-tiles accumulating in fp32 PSUM, add bias,
#            spill the h row-block to a DRAM scratch.
#   phase 2: dma_gather endpoint rows of h, write compact per-core [C', 2P]
#            result buffers.
# Phase 2 overlaps phase 1: the host buckets each stream-pair's queries by
# max(s, e) // 512, so a bucket's gathers only depend on the first few h
# row-blocks (explicit add_dep_helper edges onto an untracked DRAM scratch)
# and can stream while later row-blocks are still in the matmul.
import numpy as np

B, S, D, P = 8, 2048, 1024, 256
NQ = 8192
NCORES = 8
KB = D // 128          # contraction k-blocks
MB = S // 128          # row blocks of the batch slice

# h row-group boundaries for gather bucketing: a query belongs to the first
# group whose bound exceeds max(s, e). The top 512 rows are split into two
# finer groups so most of the late gather work unlocks before the final h
# row-blocks land (the last group is the only truly serial tail).
BOUNDS = [512, 1024, 1536, 2048]
NG = len(BOUNDS)
# Only valid (e >= s) queries are routed — invalid rows of the output are
# zero and the host result buffers start zeroed. P(valid) ~ 1/2; per-group
# means for ~1100 routed queries/core are ~(34, 103, 172, 112, 129);
# capacities sit ~8 sigma above. The gather ucode scans every one of
# num_idxs slots (pads included), so capacities directly cost GpSimd time.
CAPS = [80, 176, 272, 352]           # per-group capacity (16-granular)
SIDX = [0, 80, 256, 528]             # group starts in idx space
CIDX = sum(CAPS)                     # idx slots per stream
BLKS = [(c + 127) // 128 for c in CAPS]          # output blocks per group
BSTART = [0, 1, 3, 6]                # cumulative block starts
NBLK = sum(BLKS)                     # 9 blocks
CSLOT = 128 * NBLK                   # result-buffer rows per pair
PADROW = 0             # pad slots gather row 0 (always in range; host drops)
NIDX = 4 * CIDX                      # s1 | e1 | s2 | e2

_cache = {}


def _build_nc():
    import concourse.bacc as bacc
    import concourse.mybir as mybir
    import concourse.tile as tile
    from concourse.masks import make_identity
    from concourse.tile import add_dep_helper

    f32 = mybir.dt.float32
    nc = bacc.Bacc("TRN2", target_bir_lowering=False, debug=False,
                   num_devices=NCORES)

    x = nc.dram_tensor("x", [S, D], f32, kind="ExternalInput").ap()
    w = nc.dram_tensor("w", [D, P], f32, kind="ExternalInput").ap()
    bias = nc.dram_tensor("bias", [128, P], f32, kind="ExternalInput").ap()
    idx = nc.dram_tensor("idx", [128, NIDX // 16], mybir.dt.int16,
                         kind="ExternalInput").ap()
    cnt = nc.dram_tensor("cnt", [1, 4 * NG], mybir.dt.int32,
                         kind="ExternalInput").ap()
    r1 = nc.dram_tensor("r1", [CSLOT, 2 * P], f32, kind="ExternalOutput").ap()
    r2 = nc.dram_tensor("r2", [CSLOT, 2 * P], f32, kind="ExternalOutput").ap()
    # scratch; group-g gathers declare only the h_dram[0:512*(g+1)] range
    # they can touch, so dep tracking stays minimal (add_dep edges back it up)
    h_dram = nc.dram_tensor("h_scratch", [S, P], f32).ap()

    with tile.TileContext(nc) as tc:
        with (
            tc.tile_pool(name="consts", bufs=1) as consts,
            tc.tile_pool(name="xin", bufs=6) as xin_pool,
            tc.tile_pool(name="xt", bufs=6) as xt_pool,
            tc.tile_pool(name="hsb", bufs=4) as h_pool,
            tc.tile_pool(name="gath", bufs=1) as g_pool,
            tc.tile_pool(name="pst", bufs=5, space="PSUM") as psum_t_pool,
            tc.tile_pool(name="psh", bufs=3, space="PSUM") as psum_h_pool,
        ):
            identity = consts.tile([128, 128], f32)
            make_identity(nc, identity)

            w_sb = consts.tile([128, KB, P], f32)
            nc.scalar.dma_start(w_sb, w.rearrange("(kb k) p -> k kb p", k=128))
            bias_sb = consts.tile([128, P], f32)
            nc.scalar.dma_start(bias_sb, bias)
            idx_sb = consts.tile([128, NIDX // 16], mybir.dt.int16)
            nc.scalar.dma_start(idx_sb, idx)
            cnt_sb = consts.tile([1, 4 * NG], mybir.dt.int32)
            nc.scalar.dma_start(cnt_sb, cnt)

            # phase 1: h = X @ W + b, one [128, P] row-block at a time
            h_writes = []
            for m in range(MB):
                x_sb = xin_pool.tile([128, D], f32, tag="x")
                nc.sync.dma_start(x_sb, x[m * 128:(m + 1) * 128, :])
                h_ps = psum_h_pool.tile([128, P], f32, tag="hps")
                for kb4 in range(KB // 4):
                    xt_ps = psum_t_pool.tile([128, 4, 128], f32, tag="xtps")
                    for j in range(4):
                        kb = 4 * kb4 + j
                        nc.tensor.transpose(
                            xt_ps[:, j], x_sb[:, kb * 128:(kb + 1) * 128],
                            identity)
                    xt_sb = xt_pool.tile([128, 4, 128], f32, tag="xt")
                    # keep ACT free: its HWDGE queue carries the result DMAs,
                    # and compute ops ahead of them would head-of-line block
                    nc.vector.tensor_copy(xt_sb, xt_ps)
                    for j in range(4):
                        kb = 4 * kb4 + j
                        nc.tensor.matmul(h_ps, xt_sb[:, j],
                                         w_sb[:, kb, :],
                                         start=(kb == 0), stop=(kb == KB - 1))
                h_sb = h_pool.tile([128, P], f32, tag="h")
                nc.vector.tensor_add(h_sb, h_ps, bias_sb)
                h_writes.append(
                    nc.sync.dma_start(h_dram[m * 128:(m + 1) * 128, :], h_sb))

            # phase 2: bucketed gathers; stream order s1 | e1 | s2 | e2,
            # each stream's CTOT slots grouped by pair bucket.
            from contextlib import ExitStack
            ctx_regs = ExitStack()
            SW = CIDX // 16          # idx columns per stream
            g_tiles = {}
            for g in range(NG):
                nb = BLKS[g]
                gb0 = BSTART[g]
                for st, (r, col0) in enumerate(
                        [(r1, 0), (r1, P), (r2, 0), (r2, P)]):
                    g_sb = g_pool.tile([128, nb, P], f32, tag=f"g{st}_{g}",
                                       name=f"g{st}_{g}")
                    g_tiles[(st, g)] = g_sb
                    c0 = st * SW + SIDX[g] // 16
                    c1 = c0 + CAPS[g] // 16
                    creg = ctx_regs.enter_context(
                        nc.gpsimd.register(f"cnt{st}_{g}"))
                    nc.gpsimd.reg_load(creg, cnt_sb[0:1, g * 4 + st:
                                                    g * 4 + st + 1])
                    gi = nc.gpsimd.dma_gather(
                        g_sb, h_dram[0:BOUNDS[g], :], idx_sb[:, c0:c1],
                        num_idxs=CAPS[g], num_idxs_reg=creg, elem_size=P,
                        single_packet=False)
                    for m in range(BOUNDS[g] // 128):
                        add_dep_helper(gi.ins, h_writes[m].ins,
                                       reason=f"gather g{g} reads h rows")
                    out_view = r.rearrange("(cb p) c -> p cb c", p=128)
                    nc.scalar.dma_start(
                        out_view[:, gb0:gb0 + nb, col0:col0 + P], g_sb)
            ctx_regs.close()

    nc.compile()
    return nc


def _get_nc():
    if "nc" not in _cache:
        _cache["nc"] = _build_nc()
    return _cache["nc"]


def _numpy_ref(flag, encoded_input, start_ids_1, end_ids_1, query_batch_idx,
               start_ids_2, end_ids_2, W, b):
    h = encoded_input.astype(np.float32) @ W.astype(np.float32) + \
        b.astype(np.float32)
    qb = np.asarray(query_batch_idx).astype(np.int64)

    def span(s, e):
        s = np.asarray(s).astype(np.int64)
        e = np.asarray(e).astype(np.int64)
        rep = np.concatenate([h[qb, s], h[qb, e]], axis=-1)
        return rep * (e >= s)[:, None].astype(rep.dtype)

    return span(start_ids_1, end_ids_1), span(start_ids_2, end_ids_2)


def _route_pair(s, e, sel):
    """Bucket one stream-pair's queries (global ids `sel`) by max-row group.

    Returns (slots_idx_s, slots_idx_e, order) where order[k] = original query
    id occupying padded slot position k (concatenated groups, group-padded),
    or -1 for pad slots. Raises ValueError on capacity overflow."""
    sv, ev = s[sel], e[sel]
    valid = ev >= sv
    grp = np.searchsorted(np.asarray(BOUNDS), np.maximum(sv, ev),
                          side="right")
    idx_s = np.full(CIDX, -1, np.int64)   # -1 tail pads: gather skips them
    idx_e = np.full(CIDX, -1, np.int64)
    order = np.full(CSLOT, -1, np.int64)
    cnts = np.zeros(NG, np.int64)
    for g in range(NG):
        pos = np.nonzero(valid & (grp == g))[0]
        if len(pos) > CAPS[g]:
            raise ValueError("bucket overflow")
        sl = slice(SIDX[g], SIDX[g] + len(pos))
        idx_s[sl] = sv[pos]
        idx_e[sl] = ev[pos]
        order[128 * BSTART[g]:128 * BSTART[g] + len(pos)] = sel[pos]
        if len(pos) == 0:   # keep >= 1 non-negative index per gather
            idx_s[SIDX[g]] = PADROW
            idx_e[SIDX[g]] = PADROW
        cnts[g] = max(len(pos), 1)
    return idx_s, idx_e, order, cnts


def kernel(flag, encoded_input, start_ids_1, end_ids_1, query_batch_idx,
           start_ids_2, end_ids_2, W, b):
    from concourse.bass_utils import run_bass_kernel_spmd

    x_full = np.ascontiguousarray(np.asarray(encoded_input),
                                  dtype=np.float32)
    w_np = np.ascontiguousarray(np.asarray(W), dtype=np.float32)
    b_np = np.asarray(b).astype(np.float32)
    qb = np.asarray(query_batch_idx).astype(np.int64)
    s1 = np.asarray(start_ids_1).astype(np.int64)
    e1 = np.asarray(end_ids_1).astype(np.int64)
    s2 = np.asarray(start_ids_2).astype(np.int64)
    e2 = np.asarray(end_ids_2).astype(np.int64)

    perms = [np.nonzero(qb == bb)[0] for bb in range(B)]
    in_range = (qb.min() >= 0 and qb.max() < B and
                all(a.min() >= 0 and a.max() < S for a in (s1, e1, s2, e2)))

    in_maps, orders = [], []
    try:
        if not in_range or x_full.shape != (B, S, D):
            raise ValueError("shape/range")
        bias_rep = np.ascontiguousarray(
            np.broadcast_to(b_np[None, :], (128, P)), dtype=np.float32)
        for bb in range(B):
            sel = perms[bb]
            i1s, i1e, order1, cnt1 = _route_pair(s1, e1, sel)
            i2s, i2e, order2, cnt2 = _route_pair(s2, e2, sel)
            orders.append((order1, order2))
            idx_stream = np.concatenate([i1s, i1e, i2s, i2e]).astype(np.int16)
            idx_w = idx_stream.reshape(NIDX // 16, 16).T
            idx_w = np.ascontiguousarray(np.tile(idx_w, (8, 1)))
            # cnt[g*4 + st]; streams (s1, e1) share cnt1, (s2, e2) cnt2
            cnt_np = np.zeros((1, 4 * NG), np.int32)
            for g in range(NG):
                cnt_np[0, g * 4 + 0] = cnt1[g]
                cnt_np[0, g * 4 + 1] = cnt1[g]
                cnt_np[0, g * 4 + 2] = cnt2[g]
                cnt_np[0, g * 4 + 3] = cnt2[g]
            in_maps.append({
                "x": np.ascontiguousarray(x_full[bb]),
                "w": w_np,
                "bias": bias_rep,
                "idx": idx_w,
                "cnt": cnt_np,
            })
    except ValueError:
        res1, res2 = _numpy_ref(flag, x_full, s1, e1, qb, s2, e2, w_np, b_np)
        return np.asarray(res1, np.float32), np.asarray(res2, np.float32)

    nc = _get_nc()
    out = run_bass_kernel_spmd(nc, in_maps, core_ids=list(range(NCORES)))
    _cache["last_run"] = out

    res1 = np.zeros((NQ, 2 * P), np.float32)
    res2 = np.zeros((NQ, 2 * P), np.float32)
    for bb in range(B):
        order1, order2 = orders[bb]
        real1, real2 = order1 >= 0, order2 >= 0
        res1[order1[real1]] = out.results[bb]["r1"][real1]
        res2[order2[real2]] = out.results[bb]["r2"][real2]
    return res1, res2

